# revision 20
# baseline (speedup 1.0000x reference)
"""Causal self-attention (B=2, T=2048, C=768, H=12) on 8 NeuronCores.

Sharding: zero-collective B x strided-query sharding. Core d handles
batch b = d//4 and query rows j::4 (j = d%4) in natural order, so
q-column block [32s, 32s+32) of the on-device [*, 512] query tiles holds
logical rows [128s, 128s+128). Attention step s then multiplies k-tile s
([128s,128s+128)) against the column SUFFIX [32s:512] - width 512-32s -
which is the exact causal block minimum (34 [128x128] blocks/head vs 48
for chunk-pair sharding). Columns left of 32s would be fully masked and
are simply never computed; only the first 32 columns of each step touch
the diagonal, masked by a single [128,32] tile: allow iff kk <= 4i+j,
independent of s. Each core computes K/V for the full sequence locally
(redundant but collective-free), all 12 heads for its 512 query rows,
and the output projection for its rows. Output unshard on host is a
strided row scatter.

On-device pipeline (bf16 matmul operands, fp32 PSUM accumulation):
  xT = DMA-transpose(x) spread over SP/DVE/ACT queues
  S^T[k,q] = K^T(h)^T Q^T(h) into a 2-bank [128,2,512] PSUM tile (both
  64-row head-halves) -> ONE fused exp on ACT per step (scale=1/8, no
  max-sub; |logits| <= ~20 so f32 exp is safe) -> diagonal mask multiply
  -> P^T @ [V|1] (deferred 2 steps to hide exp latency) folds the
  softmax denominator into row 64 of the PSUM accumulator -> reciprocal
  + K=1 ones-matmul broadcast -> normalize into Y^T -> projection with
  f=0..2 partials prefetched during the last pair's attention.

Engine discipline: ACT runs only Exp + 3 transposes; K/V PSUM copy-outs
run on Pool (gpsimd); DVE keeps QTt, masks, and the normalize chain.
"""

import numpy as np
import ml_dtypes

B, T, C, H, D = 2, 2048, 768, 12, 64
NCORES = 8
NSTEPS = 16        # one step per 128-wide k-tile
DEFER = 2          # PV trails the S/exp front by this many steps

_CACHE = {}


def _build_program(with_bias=True):
    import os
    import concourse.bass as bass
    import concourse.bacc as bacc
    import concourse.mybir as mybir
    import concourse.tile as tile

    F32 = mybir.dt.float32
    BF16 = mybir.dt.bfloat16
    AF = mybir.ActivationFunctionType

    nc = bacc.Bacc()
    xbf = nc.declare_dram_parameter("xbf", [T, C], BF16, isOutput=False)
    xqbf = nc.declare_dram_parameter("xqbf", [512, C], BF16, isOutput=False)
    wqkv = nc.declare_dram_parameter("wqkv", [C, 3 * C], BF16, isOutput=False)
    wproj = nc.declare_dram_parameter("wproj", [C, C], BF16, isOutput=False)
    bqkv = nc.declare_dram_parameter("bqkv", [128, 18], F32, isOutput=False)
    bproj = nc.declare_dram_parameter("bproj", [1, C], F32, isOutput=False)
    masks = nc.declare_dram_parameter("masks", [128, 2, 32], BF16,
                                      isOutput=False)
    z_out = nc.declare_dram_parameter("z", [512, C], F32, isOutput=True)

    CT = C // 128            # 6 c-tiles
    TT = T // 128            # 16 t-tiles
    QN = 512                 # own query rows

    with tile.TileContext(nc) as tc:
        with tc.tile_pool(name="const", bufs=1) as constp, \
             tc.tile_pool(name="data", bufs=1) as datap, \
             tc.tile_pool(name="pt", bufs=4) as ptp, \
             tc.tile_pool(name="small", bufs=2) as smallp, \
             tc.tile_pool(name="zs", bufs=4) as zsp, \
             tc.tile_pool(name="ps", bufs=2, space="PSUM") as psp, \
             tc.tile_pool(name="pso", bufs=2, space="PSUM") as psop:

            # ---- constants ------------------------------------------------
            masks_s = constp.tile([128, 2, 32], BF16, tag="masks")
            bqkv_s = constp.tile([128, 18], F32, tag="bqkv")
            bpb = constp.tile([128, C], F32, tag="bpb")
            ones1 = constp.tile([1, 64], BF16, tag="ones1")
            nc.vector.memset(ones1, 1.0)
            wq_s = constp.tile([128, CT, 3 * C], BF16, tag="wqkv")
            wp_s = constp.tile([128, CT, C], BF16, tag="wproj")
            w_ap = wqkv[:, :]
            wp_ap = wproj[:, :]
            bp_ap = bproj[:, :]

            # ---- persistent tiles -----------------------------------------
            xT = [datap.tile([128, T], BF16, tag=f"xT{c}", name=f"xT{c}")
                  for c in range(CT)]
            xqT = [datap.tile([128, QN], BF16, tag=f"xqT{c}", name=f"xqT{c}")
                   for c in range(CT)]
            KT = [datap.tile([128, T], BF16, tag=f"KT{m}", name=f"KT{m}")
                  for m in range(CT)]
            QTt = [datap.tile([128, QN], BF16, tag=f"QT{m}", name=f"QT{m}")
                   for m in range(CT)]
            V = [datap.tile([128, H, D + 1], BF16, tag=f"V{t}", name=f"V{t}")
                 for t in range(TT)]
            YT = [datap.tile([128, QN], BF16, tag=f"YT{m}", name=f"YT{m}")
                  for m in range(CT)]

            # ---- phase A: startup DMAs spread across queues ---------------
            # sync(SP):   V-weight half 0, xT0, xT1, xqT0-2
            # scalar(ACT): xT3-5, xT2, xqT3-5
            # gpsimd(Pool/SWDGE): V-weight half 1, masks, K/Q weights,
            #                     proj weight, biases
            for c in range(3):
                nc.sync.dma_start_transpose(
                    out=xqT[c], in_=xqbf[:, 128 * c:128 * (c + 1)])
            for c in range(3, CT):
                nc.scalar.dma_start_transpose(
                    out=xqT[c], in_=xqbf[:, 128 * c:128 * (c + 1)])
            nc.gpsimd.dma_start(
                out=wq_s[:, :, 0:C],
                in_=bass.AP(tensor=w_ap.tensor, offset=w_ap.offset,
                            ap=[[3 * C, 128], [128 * 3 * C, CT], [1, C]]),
            )
            nc.gpsimd.dma_start(
                out=wq_s[:, :, 2 * C:3 * C],
                in_=bass.AP(tensor=w_ap.tensor, offset=w_ap.offset + 2 * C,
                            ap=[[3 * C, 128], [128 * 3 * C, CT], [1, C]]),
            )
            for c, eng in ((3, nc.scalar), (0, nc.sync), (4, nc.scalar),
                           (1, nc.sync), (5, nc.scalar), (2, nc.scalar)):
                eng.dma_start_transpose(
                    out=xT[c], in_=xbf[:, 128 * c:128 * (c + 1)])
            xgate = constp.tile([1, 1], BF16, tag="xgate")
            nc.gpsimd.tensor_copy(out=xgate, in_=xT[2][0:1, 0:1])
            nc.gpsimd.tensor_copy(out=xgate, in_=xT[5][0:1, 0:1])
            nc.gpsimd.dma_start(
                out=wq_s[:, :, C:2 * C],
                in_=bass.AP(tensor=w_ap.tensor, offset=w_ap.offset + C,
                            ap=[[3 * C, 128], [128 * 3 * C, CT], [1, C]]),
            )
            nc.gpsimd.dma_start(out=masks_s, in_=masks[:, :])
            if with_bias:
                nc.gpsimd.dma_start(out=bqkv_s, in_=bqkv[:, :])
                nc.gpsimd.dma_start(
                    out=bpb,
                    in_=bass.AP(tensor=bp_ap.tensor, offset=bp_ap.offset,
                                ap=[[0, 128], bp_ap.ap[1]]),
                )

            # ---- phase Q: all Q^T projections (fills the x-DMA window) ----
            corder = (3, 0, 4, 1, 5, 2)   # by xT DMA arrival order
            for m in range(CT):
                acc = psp.tile([128, 2, 512], F32, tag="s", name="qacc")
                for ci, c in enumerate(corder):
                    nc.tensor.matmul(
                        out=acc[:, 0, :],
                        lhsT=wq_s[:, c, 128 * m:128 * (m + 1)],
                        rhs=xqT[c], start=(ci == 0), stop=(ci == CT - 1))
                if with_bias:
                    nc.vector.tensor_scalar_add(
                        QTt[m], in0=acc[:, 0, :], scalar1=bqkv_s[:, m:m + 1])
                else:
                    nc.vector.tensor_copy(out=QTt[m], in_=acc[:, 0, :])

            # ---- phase B: V projection (full T), copy-outs on Pool --------
            for t in range(TT):
                acc = psp.tile([128, 2, 512], F32, tag="s", name="vacc")
                for ci, c in enumerate(corder):
                    nc.tensor.matmul(
                        out=acc[:, 0, :],
                        lhsT=xT[c][:, 128 * t:128 * (t + 1)],
                        rhs=wq_s[:, c, 2 * C:2 * C + 512],
                        start=(ci == 0), stop=(ci == CT - 1))
                for ci, c in enumerate(corder):
                    nc.tensor.matmul(
                        out=acc[:, 1, 0:256],
                        lhsT=xT[c][:, 128 * t:128 * (t + 1)],
                        rhs=wq_s[:, c, 2 * C + 512:3 * C],
                        start=(ci == 0), stop=(ci == CT - 1))
                nc.vector.tensor_copy(out=V[t][:, 0:8, 0:D], in_=acc[:, 0, :])
                nc.vector.tensor_copy(out=V[t][:, 8:12, 0:D],
                                      in_=acc[:, 1, 0:256])
                nc.vector.memset(V[t][:, :, D:D + 1], 1.0)

            scale = 1.0 / float(np.sqrt(D))

            def norm_front(pots):
                """Reciprocal of the folded denominators (row 64)."""
                outs = []
                for i in range(2):
                    rec = smallp.tile([1, QN], F32, tag=f"rec{i}",
                                      name=f"rec{i}")
                    nc.vector.reciprocal(out=rec, in_=pots[i][64:65, 0:QN])
                    recbf = smallp.tile([1, QN], BF16, tag=f"recbf{i}",
                                        name=f"recbf{i}")
                    nc.vector.tensor_copy(out=recbf, in_=rec)
                    outs.append(recbf)
                return outs

            def norm_back(pm, pots, recbfs):
                """Broadcast recip via K=1 matmul, normalize into YT."""
                for i in range(2):
                    po = 64 * i
                    rt = psp.tile([128, 2, 512], F32, tag="s", name="recb")
                    nc.tensor.matmul(out=rt[0:64, 0, :], lhsT=ones1,
                                     rhs=recbfs[i], start=True, stop=True)
                    rsb = smallp.tile([64, QN], F32, tag=f"rsb{i}",
                                      name=f"rsb{i}")
                    nc.vector.tensor_copy(out=rsb, in_=rt[0:64, 0, :])
                    ysl = YT[pm][po:po + 64, 0:QN]
                    nc.vector.tensor_mul(ysl, pots[i][0:64, 0:QN], rsb)
                    if with_bias:
                        nc.vector.tensor_scalar_add(
                            ysl, in0=ysl,
                            scalar1=bqkv_s[po:po + 64, 12 + pm:13 + pm])

            # ---- phase C: per-pair KT + attention -------------------------
            pending = None       # (m, ots) awaiting normalize
            recbfs = None
            drain = None         # leftover PV entries of the previous pair
            zt1 = []             # prefetched f=0..2 projection partials
            for m in range(CT):
                if m == 1:
                    # deferred: keep this transfer out of the startup DMA
                    # contention window (first needed at the projection tail)
                    nc.gpsimd.dma_start(
                        out=wp_s,
                        in_=bass.AP(tensor=wp_ap.tensor, offset=wp_ap.offset,
                                    ap=[[C, 128], [128 * C, CT],
                                        wp_ap.ap[1]]),
                    )
                for n2 in range(2):
                    acc = psp.tile([128, 2, 512], F32, tag="s", name="kacc")
                    for half in range(2):
                        n = 2 * n2 + half
                        for ci, c in enumerate(corder):
                            nc.tensor.matmul(
                                out=acc[:, half, :],
                                lhsT=wq_s[:, c, C + 128 * m:C + 128 * (m + 1)],
                                rhs=xT[c][:, 512 * n:512 * (n + 1)],
                                start=(ci == 0), stop=(ci == CT - 1))
                        if with_bias:
                            nc.vector.tensor_scalar_add(
                                KT[m][:, 512 * n:512 * (n + 1)],
                                in0=acc[:, half, :],
                                scalar1=bqkv_s[:, 6 + m:7 + m])
                        else:
                            nc.vector.tensor_copy(
                                out=KT[m][:, 512 * n:512 * (n + 1)],
                                in_=acc[:, half, :])
                if drain is not None:
                    for ent in drain[0]:
                        drain[1](ent)
                    drain = None
                if pending is not None:
                    recbfs = norm_front(pending[1])

                ots = [psop.tile([65, QN], F32, tag=f"ot{i}",
                                 name=f"ot{i}") for i in range(2)]

                def emit_pv(ent, ots=ots, m=m):
                    pt_, s_, w_ = ent
                    for i in range(2):
                        nc.tensor.matmul(
                            out=ots[i][:, 32 * s_:512],
                            lhsT=V[s_][:, 2 * m + i, :],
                            rhs=pt_[:, i, 0:w_],
                            start=(s_ == 0), stop=(s_ == NSTEPS - 1),
                            skip_group_check=True)

                pvq = []
                pack = None   # steps 11-15 (480 cols total) share one tile
                pcol = {11: 0, 12: 160, 13: 288, 14: 384, 15: 448}
                for s in range(NSTEPS):
                    w = 512 - 32 * s
                    if s < 11:
                        sps = psp.tile([128, 2, 512], F32, tag="s",
                                       name="sps")
                        c0 = 0
                    else:
                        if pack is None:
                            pack = psp.tile([128, 2, 512], F32, tag="s",
                                            name="spsp")
                        sps = pack
                        c0 = pcol[s]
                    for i, po in ((0, 0), (1, 64)):
                        nc.tensor.matmul(
                            out=sps[:, i, c0:c0 + w],
                            lhsT=KT[m][po:po + 64, 128 * s:128 * (s + 1)],
                            rhs=QTt[m][po:po + 64, 32 * s:512],
                            start=True, stop=True)
                    pt = ptp.tile([128, 2, 512], BF16, tag="pt", name="pt")
                    nc.scalar.activation(out=pt[:, :, 0:w],
                                         in_=sps[:, :, c0:c0 + w],
                                         func=AF.Exp, scale=scale)
                    nc.vector.tensor_mul(pt[:, :, 0:32], pt[:, :, 0:32],
                                         masks_s)
                    pvq.append((pt, s, w))
                    if len(pvq) > DEFER:
                        emit_pv(pvq.pop(0))
                    if s == 1 and pending is not None:
                        norm_back(pending[0], pending[1], recbfs)
                        pending = None
                    if m == CT - 1 and s in (3, 5, 7, 9):
                        # prefetch projection partials f=0..2 for one q-tile
                        qm = (s - 3) // 2
                        pacc = psp.tile([128, 2, 512], F32, tag="s",
                                        name="pacc")
                        for f in range(3):
                            nc.tensor.matmul(
                                out=pacc[:, 0, :],
                                lhsT=YT[f][:, 128 * qm:128 * (qm + 1)],
                                rhs=wp_s[:, f, 0:512],
                                start=(f == 0), stop=(f == 2))
                        for f in range(3):
                            nc.tensor.matmul(
                                out=pacc[:, 1, 0:256],
                                lhsT=YT[f][:, 128 * qm:128 * (qm + 1)],
                                rhs=wp_s[:, f, 512:C],
                                start=(f == 0), stop=(f == 2))
                        zt = zsp.tile([128, 2, 512], F32, tag="zt1",
                                      name=f"zt1_{qm}")
                        nc.vector.tensor_copy(out=zt[:, 0, :],
                                              in_=pacc[:, 0, :])
                        nc.vector.tensor_copy(out=zt[:, 1, 0:256],
                                              in_=pacc[:, 1, 0:256])
                        zt1.append(zt)
                drain = (pvq, emit_pv)
                pending = (m, ots)

            for ent in drain[0]:
                drain[1](ent)
            recbfs = norm_front(pending[1])
            norm_back(pending[0], pending[1], recbfs)

            # ---- phase D: projection tail (f=3..5) + output ---------------
            zq = (nc.sync, nc.scalar)
            for qm in range(QN // 128):
                pacc = psp.tile([128, 2, 512], F32, tag="s", name="pacc2")
                for f in range(3, CT):
                    nc.tensor.matmul(
                        out=pacc[:, 0, :],
                        lhsT=YT[f][:, 128 * qm:128 * (qm + 1)],
                        rhs=wp_s[:, f, 0:512],
                        start=(f == 3), stop=(f == CT - 1))
                for f in range(3, CT):
                    nc.tensor.matmul(
                        out=pacc[:, 1, 0:256],
                        lhsT=YT[f][:, 128 * qm:128 * (qm + 1)],
                        rhs=wp_s[:, f, 512:C],
                        start=(f == 3), stop=(f == CT - 1))
                zt = zt1[qm]
                eng = zq[qm % len(zq)]
                nc.vector.tensor_add(zt[:, 0, :], pacc[:, 0, :], zt[:, 0, :])
                if with_bias:
                    nc.vector.tensor_add(zt[:, 0, :], zt[:, 0, :],
                                         bpb[:, 0:512])
                eng.dma_start(
                    out=z_out[128 * qm:128 * (qm + 1), 0:512],
                    in_=zt[:, 0, :])
                nc.vector.tensor_add(zt[:, 1, 0:256], pacc[:, 1, 0:256],
                                     zt[:, 1, 0:256])
                if with_bias:
                    nc.vector.tensor_add(zt[:, 1, 0:256], zt[:, 1, 0:256],
                                         bpb[:, 512:C])
                eng.dma_start(
                    out=z_out[128 * qm:128 * (qm + 1), 512:C],
                    in_=zt[:, 1, 0:256])

    nc.finalize()
    return nc


def _prep_inputs(x, W_qkv, b_qkv, W_proj, b_proj):
    bf16 = ml_dtypes.bfloat16
    x = np.ascontiguousarray(np.asarray(x, dtype=np.float32))
    W_qkv = np.asarray(W_qkv, dtype=np.float32)
    b_qkv = np.asarray(b_qkv, dtype=np.float32)
    W_proj = np.asarray(W_proj, dtype=np.float32)
    b_proj = np.asarray(b_proj, dtype=np.float32)

    wqkv_b = np.ascontiguousarray(W_qkv.astype(bf16))
    wproj_b = np.ascontiguousarray(W_proj.astype(bf16))
    # b_qkv [2304] -> [128, 18] with [p, m] = b[128m + p]
    bqkv_t = np.ascontiguousarray(b_qkv.reshape(18, 128).T)
    bproj_t = np.ascontiguousarray(b_proj.reshape(1, C))
    xb = [np.ascontiguousarray(x[b].astype(bf16)) for b in range(B)]

    in_maps = []
    kk = np.arange(128)[:, None]
    ii = np.arange(32)[None, :]
    for d in range(NCORES):
        b, j = d // 4, d % 4
        xq = np.ascontiguousarray(xb[b][j::4])
        # Diagonal-band mask, identical for every step s: on-device column
        # i of step s holds logical row q = 128s + 4i + j while the k-tile
        # row kk is k = 128s + kk, so allow iff kk <= 4i + j. Stored twice
        # (once per 64-row head half of the fused [128,2,512] step tile).
        m1 = (kk <= 4 * ii + j).astype(np.float32)
        m = np.stack([m1, m1], axis=1)
        in_maps.append({
            "xbf": xb[b],
            "xqbf": xq,
            "wqkv": wqkv_b,
            "wproj": wproj_b,
            "bqkv": bqkv_t,
            "bproj": bproj_t,
            "masks": np.ascontiguousarray(m.astype(bf16)),
        })
    return in_maps


def kernel(x, W_qkv, b_qkv, W_proj, b_proj):
    import os
    from concourse.bass_utils import run_bass_kernel_spmd

    in_maps = _prep_inputs(x, W_qkv, b_qkv, W_proj, b_proj)
    with_bias = bool(np.any(np.asarray(b_qkv)) or np.any(np.asarray(b_proj)))
    key = f"nc{with_bias}"
    if key not in _CACHE:
        _CACHE[key] = _build_program(with_bias)
    nc = _CACHE[key]
    res = run_bass_kernel_spmd(nc, in_maps, list(range(NCORES)),
                               trace=os.environ.get("KTRACE", "") == "1")
    _CACHE["last_result"] = res

    out = np.empty((B, T, C), dtype=np.float32)
    for d in range(NCORES):
        b, j = d // 4, d % 4
        out[b, j::4] = np.asarray(res.results[d]["z"])
    return out
